# revision 18
# baseline (speedup 1.0000x reference)
"""GQA multi-head attention (B=1, S=4096, E=2048, H=16, HK=4, D=128) on 8 trn2
NeuronCores.

Sharding: tensor-parallel over query heads — 2 q-heads per core, each core
also computes the kv head its q-heads attend to (each kv head is replicated
on the 2 cores that need it). Each core produces a partial output
y_c = attn_c @ Wo_c and the host sums the 8 partials during unsharding
(so the device program needs no collectives).

Device-side dataflow per core (matmul inputs fp16, accumulation fp32):
  xT [E,S] -> qT [D,h,S], kT [D,S] (transposed projections), v [S,D]
  scoresT[t,sq] = (kT chunk as lhsT).T @ qT      (t-chunk on partitions)
  pT = exp(scoresT/sqrt(D)) via ACT -> fp16
  outT[d,sq] accumulated over t-chunks: lhsT=v[t,d], rhs=pT[t,sq]
  rowsums: DVE adds over t-chunks, then ones-matmul partition-sum+broadcast
  attnT = outT * (1/rowsum); o_proj: y[s,e] = (attnT as lhsT).T @ WoT

PSUM (8 banks) splits phase-wise: phase A {rot:5x[128,512], vt:1, qa:2},
phase B {sps:2x[128,1024], osum:2, oproj:2}. The attention loop is one
global software pipeline over pair-steps g=(qb,h,p) (a pair = two 128-row
t-chunks): QK(g+1) and exp(g+1) are emitted before PV(g), so the [128,1024]
ACT exp (~920ns < the PE pair + fillers) stays hidden. Independent matmuls
(output projection of the previous query block, late q-projection groups,
rowsum matmuls) fill the PE slack: o_proj closures pop from a FIFO paced
one per two steps (held 6 steps after each batch so the aT writes they
read are emitted first — emission order is semantic order per region);
q-groups and finish_head run inline at fixed steps, atomic on the shared
oproj psum tag so at most one long-lived allocation is ever open.
"""
import math
import numpy as np
from contextlib import ExitStack
from collections import deque

import concourse.bass as bass
import concourse.mybir as mybir
from concourse import tile
from concourse import bass_utils
from concourse.masks import make_identity

B, S, E = 1, 4096, 2048
H, HK, D = 16, 4, 128
N_CORES = 8
HPC = H // N_CORES          # q heads per core
QDIM = HPC * D              # 256
EC = E // 128               # e-chunks
SB = 512                    # column block
NSB = S // SB
TC = S // 128               # t-chunks
SCALE = 1.0 / math.sqrt(D)
FP16 = mybir.dt.float16
FP32 = mybir.dt.float32


def _split_sync_waits(nc, cap=1):
    """This container's walrus build rejects instructions carrying more than
    ~1 sync-wait (codegen 'Too many sync wait commands'). Post-pass over the
    scheduled BIR: for any instruction with >cap waits, hoist the excess onto
    same-engine NOPs inserted immediately before it (same block, so per-engine
    program order — and therefore semantics — is preserved)."""
    n = 0
    for fn in nc.m.functions:
        for blk in fn.blocks:
            il = blk.instructions
            i = 0
            while i < len(il):
                inst = il[i]
                si = getattr(inst, "sync_info", None)
                if si is not None and len(si.on_wait) > cap:
                    waits = list(si.on_wait)
                    si.on_wait = waits[-cap:]
                    extras = []
                    for w in waits[:-cap]:
                        nop = mybir.InstNoOp(name=f"I-waitfix-{n}", ins=[], outs=[])
                        n += 1
                        nop.engine = inst.engine
                        nop.sync_info = mybir.SyncInfo(on_wait=[w], on_update=[])
                        extras.append(nop)
                    il[i:i] = extras
                    i += len(extras)
                i += 1
    return n


XTW = 2048                  # xt tile width (half of S per tile)
NHALF = S // XTW            # 2 halves


def _emit_program(nc, tc, aps, weights, r):
    """Emit one full forward pass. `r` suffixes pool/tile names so the
    program can be repeated for timing calibration."""
    xT, y = aps
    wq_sb, wk_sb, wv_sb, wo_sb, ones_sb, ident_sb = weights

    big = tc.alloc_tile_pool(name=f"big{r}", bufs=1)
    qT_sb = big.tile([128, HPC, S], FP16, name=f"qT{r}")   # [d, h, s]
    kT_sb = big.tile([128, S], FP16, name=f"kT{r}")        # [d, t]
    v_sb = big.tile([128, S], FP16, name=f"v{r}")          # [t%128, tc*128+d]
    aT_sb = big.tile([128, HPC, S], FP16, name=f"aT{r}")   # [d, h, s]

    with ExitStack() as ctx:
        xpool = ctx.enter_context(tc.tile_pool(name=f"xpool{r}", bufs=24))
        xt_tiles = {}

        def load_half(half):
            for ec in range(EC):
                t = xpool.tile([128, XTW], FP16,
                               name=f"xt{r}_{half}_{ec}", tag="xt")
                nc.sync.dma_start(
                    t[:], xT[ec * 128:(ec + 1) * 128,
                             half * XTW:(half + 1) * XTW])
                xt_tiles[(half, ec)] = t

        def xt_slice(sb, ec, width=SB, sub=0):
            half, off = divmod(sb * SB + sub, XTW)
            return xt_tiles[(half, ec)][:, off:off + width]

        # ---- Phase A: k/v/q projections (x streamed from HBM exactly once).
        if r == 0:
            wdmas = _WEIGHT_DMAS.pop(0)
            kv_dmas = wdmas["kv"]       # list of (dst_ap, src_ap)
            q_dmas = wdmas["q"]
            late_dmas = wdmas["o"]
        else:
            kv_dmas, q_dmas, late_dmas = [], [], []
        kv_sched = {0: [0], 2: [1]}
        for ec in range(EC):
            if kv_dmas:
                for i in kv_sched.get(ec, ()):
                    nc.sync.dma_start(*kv_dmas[i])
            t = xpool.tile([128, XTW], FP16, name=f"xt{r}_0_{ec}", tag="xt")
            nc.sync.dma_start(t[:], xT[ec * 128:(ec + 1) * 128, 0:XTW])
            xt_tiles[(0, ec)] = t
        for dst, src in q_dmas:
            nc.sync.dma_start(dst, src)

        with ExitStack() as ctxA:
            psA = ctxA.enter_context(
                tc.tile_pool(name=f"psA{r}", bufs=1, space="PSUM"))
            vtp = ctxA.enter_context(tc.tile_pool(name=f"vtp{r}", bufs=4))

            def q_group(qb, ic):
                """One q-projection accumulation group (16 matmuls+evict)."""
                q_ps = psA.tile([128, SB], FP32, name=f"qpsA{r}_{qb}_{ic}",
                                tag="qa", bufs=2)
                for ec in range(EC):
                    nc.tensor.matmul(
                        q_ps[:],
                        wq_sb[:, ec, ic * 128:(ic + 1) * 128],
                        xt_slice(qb, ec),
                        start=(ec == 0), stop=(ec == EC - 1))
                nc.vector.tensor_copy(
                    qT_sb[:, ic, qb * SB:(qb + 1) * SB], q_ps[:])

            for sbp in range(NSB // 2):
                if sbp == 1:
                    load_half(1)
                if sbp == 3:
                    for dst, src in late_dmas:
                        nc.sync.dma_start(dst, src)
                sb0, sb1 = 2 * sbp, 2 * sbp + 1
                # From sbp>=1 all weights are resident, so run ec-major: the
                # four k/v accumulation groups stay open together and each
                # arriving xt tile feeds 4 matmuls at once (k-only order
                # starves the in-order PE one matmul per tile when the x
                # stream is still ahead of compute). At sbp==0 the wv DMA is
                # still in flight, so keep k-then-v order there.
                k_ps = {sb: psA.tile([128, SB], FP32, name=f"kps{r}_{sb}",
                                     tag="rot", bufs=5) for sb in (sb0, sb1)}
                if sbp == 0:
                    for sb in (sb0, sb1):
                        for ec in range(EC):
                            nc.tensor.matmul(
                                k_ps[sb][:], wk_sb[:, ec, :],
                                xt_slice(sb, ec),
                                start=(ec == 0), stop=(ec == EC - 1))
                    vT_ps = {sb: psA.tile([128, SB], FP32,
                                          name=f"vtps{r}_{sb}",
                                          tag="rot", bufs=5)
                             for sb in (sb0, sb1)}
                    for sb in (sb0, sb1):
                        for ec in range(EC):
                            nc.tensor.matmul(
                                vT_ps[sb][:], wv_sb[:, ec, :],
                                xt_slice(sb, ec),
                                start=(ec == 0), stop=(ec == EC - 1))
                else:
                    vT_ps = {sb: psA.tile([128, SB], FP32,
                                          name=f"vtps{r}_{sb}",
                                          tag="rot", bufs=5)
                             for sb in (sb0, sb1)}
                    for ec in range(EC):
                        for sb in (sb0, sb1):
                            nc.tensor.matmul(
                                k_ps[sb][:], wk_sb[:, ec, :],
                                xt_slice(sb, ec),
                                start=(ec == 0), stop=(ec == EC - 1))
                        for sb in (sb0, sb1):
                            nc.tensor.matmul(
                                vT_ps[sb][:], wv_sb[:, ec, :],
                                xt_slice(sb, ec),
                                start=(ec == 0), stop=(ec == EC - 1))
                vst = {}
                for sb in (sb0, sb1):
                    nc.vector.tensor_copy(
                        kT_sb[:, sb * SB:(sb + 1) * SB], k_ps[sb][:])
                    vst[sb] = vtp.tile([128, SB], FP16, name=f"vst{r}_{sb}",
                                       tag="vst")
                    nc.scalar.copy(vst[sb][:], vT_ps[sb][:])
                if sbp < 2:
                    q_group(sb0, 0)
                    q_group(sb0, 1)
                vt_ps = psA.tile([128, 2 * SB], FP16, name=f"vtt{r}_{sbp}",
                                 tag="vt", bufs=1)
                for j in range(2 * SB // 128):
                    src = vst[sb0 if j < 4 else sb1]
                    nc.tensor.transpose(vt_ps[:, j * 128:(j + 1) * 128],
                                        src[:, (j % 4) * 128:(j % 4 + 1) * 128],
                                        ident_sb[:])
                nc.scalar.copy(v_sb[:, sb0 * SB:(sb0 + 2) * SB], vt_ps[:])
                if sbp < 2:
                    q_group(sb1, 0)
                    q_group(sb1, 1)

        # ---- Phase B: attention as one global pipeline over (qb, h, pair) --
        # A "pair" step covers two 128-row t-chunks: QK is two N=512 matmuls
        # into one 2-bank [128,1024] psum tile, exp is one [128,1024] ACT op
        # (wider ACT ops amortize the per-instruction overhead; ACT per pair
        # = ~920ns < PE per pair = ~850ns + fillers, so PE stays the pacer).
        # PSUM: sps 2x2 banks, osum 2, oproj 2 (o_proj fillers double-
        # buffered; q-groups and rowsum matmuls share the oproj tag as
        # atomic closures so at most one long-lived allocation is open).
        with ExitStack() as ctxB:
            psB = ctxB.enter_context(
                tc.tile_pool(name=f"psB{r}", bufs=1, space="PSUM"))
            ptp = ctxB.enter_context(tc.tile_pool(name=f"ptp{r}", bufs=5))
            accp = ctxB.enter_context(tc.tile_pool(name=f"accp{r}", bufs=3))
            rcp = ctxB.enter_context(tc.tile_pool(name=f"rcp{r}", bufs=2))
            y_sbp = ctxB.enter_context(tc.tile_pool(name=f"y_sbp{r}", bufs=2))

            fillers = deque()
            n_y = [0]

            def q_group_closure(qb, ic):
                """Whole q-projection group (16 matmuls + evict), atomic on
                the shared oproj psum tag."""
                def emit():
                    q_ps = psB.tile([128, SB], FP32, name=f"qpsB{r}_{qb}_{ic}",
                                    tag="oproj", bufs=2)
                    for ec in range(EC):
                        nc.tensor.matmul(
                            q_ps[:],
                            wq_sb[:, ec, ic * 128:(ic + 1) * 128],
                            xt_slice(qb, ec),
                            start=(ec == 0), stop=(ec == EC - 1))
                    nc.vector.tensor_copy(
                        qT_sb[:, ic, qb * SB:(qb + 1) * SB], q_ps[:])
                return emit

            def o_proj_closures(qb, alt_engines=False):
                """One closure per (sc, eb): 2 matmuls + evict; DMA per sc.
                With alt_engines (used for the drain tail, where nothing else
                runs), evictions alternate DVE/ACT so neither engine's
                serialized copies gate the matmul stream."""
                cls = []
                for sc in range(qb * (SB // 128), (qb + 1) * (SB // 128)):
                    y_t = y_sbp.tile([128, E], FP16, name=f"ysb{r}_{sc}",
                                     tag="ysb")
                    for eb in range(E // SB):
                        def mk(sc, eb, y_t):
                            def emit():
                                y_ps = psB.tile([128, SB], FP32,
                                                name=f"yps{r}_{sc}_{eb}",
                                                tag="oproj", bufs=2)
                                for h in range(HPC):
                                    nc.tensor.matmul(
                                        y_ps[:],
                                        aT_sb[:, h, sc * 128:(sc + 1) * 128],
                                        wo_sb[:, h, eb * SB:(eb + 1) * SB],
                                        start=(h == 0), stop=(h == HPC - 1))
                                if alt_engines and (sc * 4 + eb) % 2:
                                    nc.scalar.copy(
                                        y_t[:, eb * SB:(eb + 1) * SB], y_ps[:])
                                else:
                                    nc.vector.tensor_copy(
                                        y_t[:, eb * SB:(eb + 1) * SB], y_ps[:])
                                n_y[0] += 1
                                if eb == E // SB - 1:
                                    nc.sync.dma_start(
                                        y[sc * 128:(sc + 1) * 128, :], y_t[:])
                            return emit
                        cls.append(mk(sc, eb, y_t))
                return cls

            def finish_head(qb, h, o_ps, sums):
                def emit():
                    sums_ps = psB.tile([128, SB], FP32,
                                       name=f"sums{r}_{qb}_{h}", tag="oproj",
                                       bufs=2)
                    nc.tensor.matmul(sums_ps[:], ones_sb[:], sums[:],
                                     start=True, stop=True)
                    recip = rcp.tile([128, SB], FP32,
                                     name=f"recip{r}_{qb}_{h}", tag="recip")
                    nc.vector.reciprocal(recip[:], sums_ps[:])
                    nc.vector.tensor_mul(
                        aT_sb[:, h, qb * SB:(qb + 1) * SB], o_ps[:], recip[:])
                return emit

            TP = TC // 2        # pairs per head
            seq = [(qb, h, p)
                   for qb in range(NSB) for h in range(HPC) for p in range(TP)]
            n = len(seq)
            sps_tiles = [None] * 2
            pt_tiles = {}
            inline_at = {}      # pair step -> list of closures to emit there

            def qk(g):
                qb, h, p = seq[g]
                s_ps = psB.tile([128, 2 * SB], FP32, name=f"sps{r}_{g}",
                                tag="sps", bufs=2)
                for hf in range(2):
                    c = p * 2 + hf
                    nc.tensor.matmul(
                        s_ps[:, hf * SB:(hf + 1) * SB],
                        kT_sb[:, c * 128:(c + 1) * 128],
                        qT_sb[:, h, qb * SB:(qb + 1) * SB],
                        start=True, stop=True)
                sps_tiles[g % 2] = s_ps

            def exp_emit(g):
                pt = ptp.tile([128, 2 * SB], FP16, name=f"pt{r}_{g}",
                              tag="pt")
                nc.scalar.activation(
                    pt[:], sps_tiles[g % 2][:],
                    mybir.ActivationFunctionType.Exp, scale=SCALE)
                pt_tiles[g] = pt

            # Late q-projection groups, spread through phase B at fixed
            # pair-steps: group j = (qb 4+j//2, ic j%2) lands well before
            # that qb's first QK (qb4 needs step<128, qb5<160, ..., qb7<224).
            for j in range((NSB // 2) * HPC):
                qb_late = NSB // 2 + j // HPC
                inline_at.setdefault(4 + j * 24, []).append(
                    q_group_closure(qb_late, j % HPC))

            qk(0)
            exp_emit(0)
            o_ps = acc2 = pt0 = None
            tail = []
            hold = 0    # suppress o_proj pops until the qb's finish_heads
            # have been emitted (their aT writes must precede the pops'
            # aT reads in program order, or no dependency is created).
            for g in range(n):
                qb, h, p = seq[g]
                if p == 0:
                    o_ps = psB.tile([128, SB], FP32, name=f"ops{r}_{qb}_{h}",
                                    tag="osum", bufs=2)
                    acc2 = accp.tile([128, 2 * SB], FP16,
                                     name=f"acc{r}_{qb}_{h}", tag="acc")
                if g + 1 < n:
                    qk(g + 1)
                    exp_emit(g + 1)
                pt = pt_tiles.pop(g)
                for hf in range(2):
                    c = p * 2 + hf
                    nc.tensor.matmul(
                        o_ps[:], v_sb[:, c * 128:(c + 1) * 128],
                        pt[:, hf * SB:(hf + 1) * SB],
                        start=(c == 0), stop=(c == TC - 1))
                if p == 0:
                    pt0 = pt
                elif p == 1:
                    nc.vector.tensor_add(acc2[:], pt0[:], pt[:])
                else:
                    nc.vector.tensor_add(acc2[:], acc2[:], pt[:])
                for cl in inline_at.pop(g, ()):
                    cl()
                # o_proj fillers: 16 closures arrive per qb (32 pair-steps);
                # spread one per 2 steps, draining faster only on backlog.
                if hold:
                    hold -= 1
                elif fillers and (g % 2 == 1 or len(fillers) > 12):
                    fillers.popleft()()
                    if len(fillers) > 12 and fillers:
                        fillers.popleft()()
                if p == TP - 1:
                    sums = accp.tile([128, SB], FP16, name=f"hsum{r}_{qb}_{h}",
                                     tag="sums", bufs=2)
                    nc.vector.tensor_add(sums[:], acc2[:, 0:SB],
                                         acc2[:, SB:2 * SB])
                    # finish_head inline a few steps into the next head so
                    # the osum bank recycles before it's needed again.
                    inline_at.setdefault(g + 4, []).append(
                        finish_head(qb, h, o_ps, sums))
                    if h == HPC - 1:
                        if qb < NSB - 1:
                            fillers.extend(o_proj_closures(qb))
                            hold = 6
                        else:
                            tail = o_proj_closures(qb)

            for g in sorted(inline_at):
                for cl in inline_at[g]:
                    cl()
            inline_at.clear()
            while fillers:
                fillers.popleft()()
            for cl in tail:
                cl()

    big.release()


_WEIGHT_DMAS = []


def build_bass(reps=1):
    nc = bass.Bass("TRN2", target_bir_lowering=False, debug=False,
                   num_devices=N_CORES)
    xT = nc.dram_tensor("xT", [E, S], FP16, kind="ExternalInput").ap()
    wq = nc.dram_tensor("wq", [E, QDIM], FP16, kind="ExternalInput").ap()
    wk = nc.dram_tensor("wk", [E, D], FP16, kind="ExternalInput").ap()
    wv = nc.dram_tensor("wv", [E, D], FP16, kind="ExternalInput").ap()
    wo = nc.dram_tensor("wo", [QDIM, E], FP16, kind="ExternalInput").ap()
    y = nc.dram_tensor("y", [S, E], FP16, kind="ExternalOutput").ap()

    with tile.TileContext(nc) as tc, ExitStack() as ctx:
        wpool = ctx.enter_context(tc.tile_pool(name="wpool", bufs=1))
        wq_sb = wpool.tile([128, EC, QDIM], FP16)
        wk_sb = wpool.tile([128, EC, D], FP16)
        wv_sb = wpool.tile([128, EC, D], FP16)
        wo_sb = wpool.tile([128, HPC, E], FP16)
        ones_sb = wpool.tile([128, 128], FP16)
        nc.vector.memset(ones_sb[:], 1.0)
        ident_sb = wpool.tile([128, 128], FP16)
        make_identity(nc, ident_sb)
        # Weights are DMA'd in chunks so the first projection matmuls only
        # wait for the slice they read, not the whole (strided, slow) tensor.
        wk_src = wk.rearrange("(ec p) d -> p ec d", p=128)
        wv_src = wv.rearrange("(ec p) d -> p ec d", p=128)
        wq_src = wq.rearrange("(ec p) d -> p ec d", p=128)
        kv_dmas = [
            (wk_sb[:], wk_src),
            (wv_sb[:], wv_src),
        ]
        q_dmas = [
            (wq_sb[:], wq_src),
        ]
        o_dmas = [
            (wo_sb[:], wo.rearrange("(h p) e -> p h e", p=128)),
        ]
        _WEIGHT_DMAS.clear()
        _WEIGHT_DMAS.append({"kv": kv_dmas, "q": q_dmas, "o": o_dmas})

        for r in range(reps):
            _emit_program(nc, tc, (xT, y),
                          (wq_sb, wk_sb, wv_sb, wo_sb, ones_sb, ident_sb), r)

    _split_sync_waits(nc)
    return nc


def make_in_maps(x, Wq, Wk, Wv, Wo):
    """Host-side sharding: transpose/cast to fp16, slice weights per core."""
    x = np.asarray(x, dtype=np.float32).reshape(S, E)
    xT = np.ascontiguousarray(x.T).astype(np.float16)
    WqT = np.ascontiguousarray(np.asarray(Wq, dtype=np.float32).T).astype(np.float16)
    WkT = np.ascontiguousarray(np.asarray(Wk, dtype=np.float32).T).astype(np.float16)
    WvT = np.ascontiguousarray(np.asarray(Wv, dtype=np.float32).T).astype(np.float16)
    WoT = np.ascontiguousarray(np.asarray(Wo, dtype=np.float32).T).astype(np.float16)
    in_maps = []
    for c in range(N_CORES):
        g = (c * HPC) // (H // HK)      # kv head for this core's q heads
        in_maps.append({
            "xT": xT,
            "wq": np.ascontiguousarray(WqT[:, c * QDIM:(c + 1) * QDIM]),
            "wk": np.ascontiguousarray(WkT[:, g * D:(g + 1) * D]),
            "wv": np.ascontiguousarray(WvT[:, g * D:(g + 1) * D]),
            "wo": np.ascontiguousarray(WoT[c * QDIM:(c + 1) * QDIM, :]),
        })
    return in_maps


_NC_CACHE = None


def get_nc():
    global _NC_CACHE
    if _NC_CACHE is None:
        _NC_CACHE = build_bass()
    return _NC_CACHE


def kernel(x, Wq, Wk, Wv, Wo):
    nc = get_nc()
    in_maps = make_in_maps(x, Wq, Wk, Wv, Wo)
    res = bass_utils.run_bass_kernel_spmd(
        nc, in_maps, core_ids=list(range(N_CORES)))
    out = np.zeros((S, E), dtype=np.float32)
    for r in res.results:
        out += r["y"].astype(np.float32)
    return out.reshape(B, S, E)


# revision 19
# speedup vs baseline: 1.0529x; 1.0529x over previous
"""GQA multi-head attention (B=1, S=4096, E=2048, H=16, HK=4, D=128) on 8 trn2
NeuronCores.

Sharding: tensor-parallel over query heads — 2 q-heads per core, each core
also computes the kv head its q-heads attend to (each kv head is replicated
on the 2 cores that need it). Each core produces a partial output
y_c = attn_c @ Wo_c and the host sums the 8 partials during unsharding
(so the device program needs no collectives).

Device-side dataflow per core (matmul inputs fp16, accumulation fp32):
  xT [E,S] -> qT [D,h,S], kT [D,S] (transposed projections), v [S,D]
  scoresT[t,sq] = (kT chunk as lhsT).T @ qT      (t-chunk on partitions)
  pT = exp(scoresT/sqrt(D)) via ACT -> fp16
  outT[d,sq] accumulated over t-chunks: lhsT=v[t,d], rhs=pT[t,sq]
  rowsums: DVE adds over t-chunks, then ones-matmul partition-sum+broadcast
  attnT = outT * (1/rowsum); o_proj: y[s,e] = (attnT as lhsT).T @ WoT

PSUM (8 banks) splits phase-wise: phase A {rot:5x[128,512], vt:1, qa:2},
phase B {sps:2x[128,1024], osum:2, oproj:2}. The attention loop is one
global software pipeline over pair-steps g=(qb,h,p) (a pair = two 128-row
t-chunks): QK(g+1) and exp(g+1) are emitted before PV(g), so the [128,1024]
ACT exp (~920ns < the PE pair + fillers) stays hidden. Independent matmuls
(output projection of the previous query block, late q-projection groups,
rowsum matmuls) fill the PE slack: o_proj closures pop from a FIFO paced
one per two steps (held 6 steps after each batch so the aT writes they
read are emitted first — emission order is semantic order per region);
q-groups and finish_head run inline at fixed steps, atomic on the shared
oproj psum tag so at most one long-lived allocation is ever open.
"""
import math
import numpy as np
from contextlib import ExitStack
from collections import deque

import concourse.bass as bass
import concourse.mybir as mybir
from concourse import tile
from concourse import bass_utils
from concourse.masks import make_identity

B, S, E = 1, 4096, 2048
H, HK, D = 16, 4, 128
N_CORES = 8
HPC = H // N_CORES          # q heads per core
QDIM = HPC * D              # 256
EC = E // 128               # e-chunks
SB = 512                    # column block
NSB = S // SB
TC = S // 128               # t-chunks
SCALE = 1.0 / math.sqrt(D)
FP16 = mybir.dt.float16
FP32 = mybir.dt.float32


def _split_sync_waits(nc, cap=1):
    """This container's walrus build rejects instructions carrying more than
    ~1 sync-wait (codegen 'Too many sync wait commands'). Post-pass over the
    scheduled BIR: for any instruction with >cap waits, hoist the excess onto
    same-engine NOPs inserted immediately before it (same block, so per-engine
    program order — and therefore semantics — is preserved)."""
    n = 0
    for fn in nc.m.functions:
        for blk in fn.blocks:
            il = blk.instructions
            i = 0
            while i < len(il):
                inst = il[i]
                si = getattr(inst, "sync_info", None)
                if si is not None and len(si.on_wait) > cap:
                    waits = list(si.on_wait)
                    si.on_wait = waits[-cap:]
                    extras = []
                    for w in waits[:-cap]:
                        nop = mybir.InstNoOp(name=f"I-waitfix-{n}", ins=[], outs=[])
                        n += 1
                        nop.engine = inst.engine
                        nop.sync_info = mybir.SyncInfo(on_wait=[w], on_update=[])
                        extras.append(nop)
                    il[i:i] = extras
                    i += len(extras)
                i += 1
    return n


XTW = 2048                  # xt tile width (half of S per tile)
NHALF = S // XTW            # 2 halves


def _emit_program(nc, tc, aps, weights, r):
    """Emit one full forward pass. `r` suffixes pool/tile names so the
    program can be repeated for timing calibration."""
    xT, y = aps
    wq_sb, wk_sb, wv_sb, wo_sb, ones_sb, ident_sb = weights

    big = tc.alloc_tile_pool(name=f"big{r}", bufs=1)
    qT_sb = big.tile([128, HPC, S], FP16, name=f"qT{r}")   # [d, h, s]
    kT_sb = big.tile([128, S], FP16, name=f"kT{r}")        # [d, t]
    v_sb = big.tile([128, S], FP16, name=f"v{r}")          # [t%128, tc*128+d]
    aT_sb = big.tile([128, HPC, S], FP16, name=f"aT{r}")   # [d, h, s]

    with ExitStack() as ctx:
        xpool = ctx.enter_context(tc.tile_pool(name=f"xpool{r}", bufs=24))
        xt_tiles = {}

        def load_half(half):
            for ec in range(EC):
                t = xpool.tile([128, XTW], FP16,
                               name=f"xt{r}_{half}_{ec}", tag="xt")
                nc.sync.dma_start(
                    t[:], xT[ec * 128:(ec + 1) * 128,
                             half * XTW:(half + 1) * XTW])
                xt_tiles[(half, ec)] = t

        def xt_slice(sb, ec, width=SB, sub=0):
            half, off = divmod(sb * SB + sub, XTW)
            return xt_tiles[(half, ec)][:, off:off + width]

        # ---- Phase A: k/v/q projections (x streamed from HBM exactly once).
        if r == 0:
            wdmas = _WEIGHT_DMAS.pop(0)
            kv_dmas = wdmas["kv"]       # list of (dst_ap, src_ap)
            q_dmas = wdmas["q"]
            late_dmas = wdmas["o"]
        else:
            kv_dmas, q_dmas, late_dmas = [], [], []
        kv_sched = {0: [0], 2: [1]}
        for ec in range(EC):
            if kv_dmas:
                for i in kv_sched.get(ec, ()):
                    nc.sync.dma_start(*kv_dmas[i])
            t = xpool.tile([128, XTW], FP16, name=f"xt{r}_0_{ec}", tag="xt")
            nc.sync.dma_start(t[:], xT[ec * 128:(ec + 1) * 128, 0:XTW])
            xt_tiles[(0, ec)] = t
        for dst, src in q_dmas:
            nc.sync.dma_start(dst, src)

        with ExitStack() as ctxA:
            psA = ctxA.enter_context(
                tc.tile_pool(name=f"psA{r}", bufs=1, space="PSUM"))
            vtp = ctxA.enter_context(tc.tile_pool(name=f"vtp{r}", bufs=4))

            def q_group(qb, ic):
                """One q-projection accumulation group (16 matmuls+evict)."""
                q_ps = psA.tile([128, SB], FP32, name=f"qpsA{r}_{qb}_{ic}",
                                tag="qa", bufs=2)
                for ec in range(EC):
                    nc.tensor.matmul(
                        q_ps[:],
                        wq_sb[:, ec, ic * 128:(ic + 1) * 128],
                        xt_slice(qb, ec),
                        start=(ec == 0), stop=(ec == EC - 1))
                nc.vector.tensor_copy(
                    qT_sb[:, ic, qb * SB:(qb + 1) * SB], q_ps[:])

            for sbp in range(NSB // 2):
                if sbp == 1:
                    load_half(1)
                if sbp == 3:
                    for dst, src in late_dmas:
                        nc.sync.dma_start(dst, src)
                sb0, sb1 = 2 * sbp, 2 * sbp + 1
                for sb in (sb0, sb1):
                    k_ps = psA.tile([128, SB], FP32, name=f"kps{r}_{sb}",
                                    tag="rot", bufs=5)
                    for ec in range(EC):
                        nc.tensor.matmul(k_ps[:], wk_sb[:, ec, :],
                                         xt_slice(sb, ec),
                                         start=(ec == 0), stop=(ec == EC - 1))
                    nc.vector.tensor_copy(
                        kT_sb[:, sb * SB:(sb + 1) * SB], k_ps[:])
                if sbp < 2:
                    q_group(sb0, 0)
                vst = {}
                for sb in (sb0, sb1):
                    vT_ps = psA.tile([128, SB], FP32, name=f"vtps{r}_{sb}",
                                     tag="rot", bufs=5)
                    for ec in range(EC):
                        nc.tensor.matmul(vT_ps[:], wv_sb[:, ec, :],
                                         xt_slice(sb, ec),
                                         start=(ec == 0), stop=(ec == EC - 1))
                    vst[sb] = vtp.tile([128, SB], FP16, name=f"vst{r}_{sb}",
                                       tag="vst")
                    nc.scalar.copy(vst[sb][:], vT_ps[:])
                if sbp < 2:
                    q_group(sb0, 1)
                vt_ps = psA.tile([128, 2 * SB], FP16, name=f"vtt{r}_{sbp}",
                                 tag="vt", bufs=1)
                for j in range(2 * SB // 128):
                    src = vst[sb0 if j < 4 else sb1]
                    nc.tensor.transpose(vt_ps[:, j * 128:(j + 1) * 128],
                                        src[:, (j % 4) * 128:(j % 4 + 1) * 128],
                                        ident_sb[:])
                nc.scalar.copy(v_sb[:, sb0 * SB:(sb0 + 2) * SB], vt_ps[:])
                if sbp < 2:
                    q_group(sb1, 0)
                    q_group(sb1, 1)

        # ---- Phase B: attention as one global pipeline over (qb, h, pair) --
        # A "pair" step covers two 128-row t-chunks: QK is two N=512 matmuls
        # into one 2-bank [128,1024] psum tile, exp is one [128,1024] ACT op
        # (wider ACT ops amortize the per-instruction overhead; ACT per pair
        # = ~920ns < PE per pair = ~850ns + fillers, so PE stays the pacer).
        # PSUM: sps 2x2 banks, osum 2, oproj 2 (o_proj fillers double-
        # buffered; q-groups and rowsum matmuls share the oproj tag as
        # atomic closures so at most one long-lived allocation is open).
        with ExitStack() as ctxB:
            psB = ctxB.enter_context(
                tc.tile_pool(name=f"psB{r}", bufs=1, space="PSUM"))
            ptp = ctxB.enter_context(tc.tile_pool(name=f"ptp{r}", bufs=5))
            accp = ctxB.enter_context(tc.tile_pool(name=f"accp{r}", bufs=3))
            rcp = ctxB.enter_context(tc.tile_pool(name=f"rcp{r}", bufs=2))
            y_sbp = ctxB.enter_context(tc.tile_pool(name=f"y_sbp{r}", bufs=2))

            fillers = deque()
            n_y = [0]

            def q_group_closure(qb, ic):
                """Whole q-projection group (16 matmuls + evict), atomic on
                the shared oproj psum tag."""
                def emit():
                    q_ps = psB.tile([128, SB], FP32, name=f"qpsB{r}_{qb}_{ic}",
                                    tag="oproj", bufs=2)
                    for ec in range(EC):
                        nc.tensor.matmul(
                            q_ps[:],
                            wq_sb[:, ec, ic * 128:(ic + 1) * 128],
                            xt_slice(qb, ec),
                            start=(ec == 0), stop=(ec == EC - 1))
                    nc.vector.tensor_copy(
                        qT_sb[:, ic, qb * SB:(qb + 1) * SB], q_ps[:])
                return emit

            def o_proj_closures(qb, alt_engines=False):
                """One closure per (sc, eb): 2 matmuls + evict; DMA per sc.
                With alt_engines (used for the drain tail, where nothing else
                runs), evictions alternate DVE/ACT so neither engine's
                serialized copies gate the matmul stream."""
                cls = []
                for sc in range(qb * (SB // 128), (qb + 1) * (SB // 128)):
                    y_t = y_sbp.tile([128, E], FP16, name=f"ysb{r}_{sc}",
                                     tag="ysb")
                    for eb in range(E // SB):
                        def mk(sc, eb, y_t):
                            def emit():
                                y_ps = psB.tile([128, SB], FP32,
                                                name=f"yps{r}_{sc}_{eb}",
                                                tag="oproj", bufs=2)
                                for h in range(HPC):
                                    nc.tensor.matmul(
                                        y_ps[:],
                                        aT_sb[:, h, sc * 128:(sc + 1) * 128],
                                        wo_sb[:, h, eb * SB:(eb + 1) * SB],
                                        start=(h == 0), stop=(h == HPC - 1))
                                if alt_engines and (sc * 4 + eb) % 2:
                                    nc.scalar.copy(
                                        y_t[:, eb * SB:(eb + 1) * SB], y_ps[:])
                                else:
                                    nc.vector.tensor_copy(
                                        y_t[:, eb * SB:(eb + 1) * SB], y_ps[:])
                                n_y[0] += 1
                                if eb == E // SB - 1:
                                    nc.sync.dma_start(
                                        y[sc * 128:(sc + 1) * 128, :], y_t[:])
                            return emit
                        cls.append(mk(sc, eb, y_t))
                return cls

            def finish_head(qb, h, o_ps, sums):
                def emit():
                    sums_ps = psB.tile([128, SB], FP32,
                                       name=f"sums{r}_{qb}_{h}", tag="oproj",
                                       bufs=2)
                    nc.tensor.matmul(sums_ps[:], ones_sb[:], sums[:],
                                     start=True, stop=True)
                    recip = rcp.tile([128, SB], FP32,
                                     name=f"recip{r}_{qb}_{h}", tag="recip")
                    nc.vector.reciprocal(recip[:], sums_ps[:])
                    nc.vector.tensor_mul(
                        aT_sb[:, h, qb * SB:(qb + 1) * SB], o_ps[:], recip[:])
                return emit

            TP = TC // 2        # pairs per head
            seq = [(qb, h, p)
                   for qb in range(NSB) for h in range(HPC) for p in range(TP)]
            n = len(seq)
            sps_tiles = [None] * 2
            pt_tiles = {}
            inline_at = {}      # pair step -> list of closures to emit there

            def qk(g):
                qb, h, p = seq[g]
                s_ps = psB.tile([128, 2 * SB], FP32, name=f"sps{r}_{g}",
                                tag="sps", bufs=2)
                for hf in range(2):
                    c = p * 2 + hf
                    nc.tensor.matmul(
                        s_ps[:, hf * SB:(hf + 1) * SB],
                        kT_sb[:, c * 128:(c + 1) * 128],
                        qT_sb[:, h, qb * SB:(qb + 1) * SB],
                        start=True, stop=True)
                sps_tiles[g % 2] = s_ps

            def exp_emit(g):
                pt = ptp.tile([128, 2 * SB], FP16, name=f"pt{r}_{g}",
                              tag="pt")
                nc.scalar.activation(
                    pt[:], sps_tiles[g % 2][:],
                    mybir.ActivationFunctionType.Exp, scale=SCALE)
                pt_tiles[g] = pt

            # Late q-projection groups, spread through phase B at fixed
            # pair-steps: group j = (qb 4+j//2, ic j%2) lands well before
            # that qb's first QK (qb4 needs step<128, qb5<160, ..., qb7<224).
            for j in range((NSB // 2) * HPC):
                qb_late = NSB // 2 + j // HPC
                inline_at.setdefault(4 + j * 24, []).append(
                    q_group_closure(qb_late, j % HPC))

            qk(0)
            exp_emit(0)
            o_ps = acc2 = pt0 = None
            tail = []
            hold = 0    # suppress o_proj pops until the qb's finish_heads
            # have been emitted (their aT writes must precede the pops'
            # aT reads in program order, or no dependency is created).
            for g in range(n):
                qb, h, p = seq[g]
                if p == 0:
                    o_ps = psB.tile([128, SB], FP32, name=f"ops{r}_{qb}_{h}",
                                    tag="osum", bufs=2)
                    acc2 = accp.tile([128, 2 * SB], FP16,
                                     name=f"acc{r}_{qb}_{h}", tag="acc")
                if g + 1 < n:
                    qk(g + 1)
                    exp_emit(g + 1)
                pt = pt_tiles.pop(g)
                for hf in range(2):
                    c = p * 2 + hf
                    nc.tensor.matmul(
                        o_ps[:], v_sb[:, c * 128:(c + 1) * 128],
                        pt[:, hf * SB:(hf + 1) * SB],
                        start=(c == 0), stop=(c == TC - 1))
                if p == 0:
                    pt0 = pt
                elif p == 1:
                    nc.vector.tensor_add(acc2[:], pt0[:], pt[:])
                else:
                    nc.vector.tensor_add(acc2[:], acc2[:], pt[:])
                for cl in inline_at.pop(g, ()):
                    cl()
                # o_proj fillers: 16 closures arrive per qb (32 pair-steps);
                # spread one per 2 steps, draining faster only on backlog.
                if hold:
                    hold -= 1
                elif fillers and (g % 2 == 1 or len(fillers) > 12):
                    fillers.popleft()()
                    if len(fillers) > 12 and fillers:
                        fillers.popleft()()
                if p == TP - 1:
                    sums = accp.tile([128, SB], FP16, name=f"hsum{r}_{qb}_{h}",
                                     tag="sums", bufs=2)
                    nc.vector.tensor_add(sums[:], acc2[:, 0:SB],
                                         acc2[:, SB:2 * SB])
                    # finish_head inline a few steps into the next head so
                    # the osum bank recycles before it's needed again.
                    inline_at.setdefault(g + 4, []).append(
                        finish_head(qb, h, o_ps, sums))
                    if h == HPC - 1:
                        if qb < NSB - 1:
                            fillers.extend(o_proj_closures(qb))
                            hold = 6
                        else:
                            tail = o_proj_closures(qb)

            for g in sorted(inline_at):
                for cl in inline_at[g]:
                    cl()
            inline_at.clear()
            while fillers:
                fillers.popleft()()
            for cl in tail:
                cl()

    big.release()


_WEIGHT_DMAS = []


def build_bass(reps=1):
    nc = bass.Bass("TRN2", target_bir_lowering=False, debug=False,
                   num_devices=N_CORES)
    xT = nc.dram_tensor("xT", [E, S], FP16, kind="ExternalInput").ap()
    wq = nc.dram_tensor("wq", [E, QDIM], FP16, kind="ExternalInput").ap()
    wk = nc.dram_tensor("wk", [E, D], FP16, kind="ExternalInput").ap()
    wv = nc.dram_tensor("wv", [E, D], FP16, kind="ExternalInput").ap()
    wo = nc.dram_tensor("wo", [QDIM, E], FP16, kind="ExternalInput").ap()
    y = nc.dram_tensor("y", [S, E], FP16, kind="ExternalOutput").ap()

    with tile.TileContext(nc) as tc, ExitStack() as ctx:
        wpool = ctx.enter_context(tc.tile_pool(name="wpool", bufs=1))
        wq_sb = wpool.tile([128, EC, QDIM], FP16)
        wk_sb = wpool.tile([128, EC, D], FP16)
        wv_sb = wpool.tile([128, EC, D], FP16)
        wo_sb = wpool.tile([128, HPC, E], FP16)
        ones_sb = wpool.tile([128, 128], FP16)
        nc.vector.memset(ones_sb[:], 1.0)
        ident_sb = wpool.tile([128, 128], FP16)
        make_identity(nc, ident_sb)
        # Weights are DMA'd in chunks so the first projection matmuls only
        # wait for the slice they read, not the whole (strided, slow) tensor.
        wk_src = wk.rearrange("(ec p) d -> p ec d", p=128)
        wv_src = wv.rearrange("(ec p) d -> p ec d", p=128)
        wq_src = wq.rearrange("(ec p) d -> p ec d", p=128)
        kv_dmas = [
            (wk_sb[:], wk_src),
            (wv_sb[:], wv_src),
        ]
        q_dmas = [
            (wq_sb[:], wq_src),
        ]
        o_dmas = [
            (wo_sb[:], wo.rearrange("(h p) e -> p h e", p=128)),
        ]
        _WEIGHT_DMAS.clear()
        _WEIGHT_DMAS.append({"kv": kv_dmas, "q": q_dmas, "o": o_dmas})

        for r in range(reps):
            _emit_program(nc, tc, (xT, y),
                          (wq_sb, wk_sb, wv_sb, wo_sb, ones_sb, ident_sb), r)

    _split_sync_waits(nc)
    return nc


def make_in_maps(x, Wq, Wk, Wv, Wo):
    """Host-side sharding: transpose/cast to fp16, slice weights per core."""
    x = np.asarray(x, dtype=np.float32).reshape(S, E)
    xT = np.ascontiguousarray(x.T).astype(np.float16)
    WqT = np.ascontiguousarray(np.asarray(Wq, dtype=np.float32).T).astype(np.float16)
    WkT = np.ascontiguousarray(np.asarray(Wk, dtype=np.float32).T).astype(np.float16)
    WvT = np.ascontiguousarray(np.asarray(Wv, dtype=np.float32).T).astype(np.float16)
    WoT = np.ascontiguousarray(np.asarray(Wo, dtype=np.float32).T).astype(np.float16)
    in_maps = []
    for c in range(N_CORES):
        g = (c * HPC) // (H // HK)      # kv head for this core's q heads
        in_maps.append({
            "xT": xT,
            "wq": np.ascontiguousarray(WqT[:, c * QDIM:(c + 1) * QDIM]),
            "wk": np.ascontiguousarray(WkT[:, g * D:(g + 1) * D]),
            "wv": np.ascontiguousarray(WvT[:, g * D:(g + 1) * D]),
            "wo": np.ascontiguousarray(WoT[c * QDIM:(c + 1) * QDIM, :]),
        })
    return in_maps


_NC_CACHE = None


def get_nc():
    global _NC_CACHE
    if _NC_CACHE is None:
        _NC_CACHE = build_bass()
    return _NC_CACHE


def kernel(x, Wq, Wk, Wv, Wo):
    nc = get_nc()
    in_maps = make_in_maps(x, Wq, Wk, Wv, Wo)
    res = bass_utils.run_bass_kernel_spmd(
        nc, in_maps, core_ids=list(range(N_CORES)))
    out = np.zeros((S, E), dtype=np.float32)
    for r in res.results:
        out += r["y"].astype(np.float32)
    return out.reshape(B, S, E)
